# revision 2
# baseline (speedup 1.0000x reference)
"""LSTM layer kernel for Trainium2 (8 NeuronCores, Bass/Tile).

Reference computation (fp32):
    z = concat([x, h], axis=1)                 # [B, IN+OUT]
    f = sigmoid(z @ w_f + b_f)
    i = sigmoid(z @ w_i + b_i)
    g = tanh   (z @ w_c + b_c)
    o = sigmoid(z @ w_o + b_o)
    c_new = c * f + i * g
    h_new = tanh(c_new) * o                    # [B, OUT]

Shapes: B=4096, IN=OUT=1024, K=IN+OUT=2048.

Sharding (8 cores): 2-D grid, 4 batch-groups x 2 output-column-groups.
Core (i, j) computes h_new[i*1024:(i+1)*1024, j*512:(j+1)*512].
Per-core HBM traffic (bf16 matmul operands): 4 MiB zT + 8 MiB weights
+ 2 MiB cT + 2 MiB out = 16 MiB.  No collectives.

Device layout: everything is transposed so the contraction dim (k) and the
output-channel dim (o) sit on SBUF partitions:
    zT [2048, 1024]  (k, b)       - moving operand (bf16)
    w  [2048, 4, 4, 128] (k, oc, gate, p) - stationary operand, fused across
         gates so one DMA covers all four gates of an o-chunk
    out = w.T @ zT -> psum [o, b], so the per-o bias is a per-partition
    ACT bias and sigmoid/tanh run straight out of PSUM.
The host pre-transposes the x/h/c shards (converting matmul operands to
bf16; z@w in bf16 keeps the final rel err ~2e-3, far under the 2e-2 gate)
and transposes the per-core h_newT shards back when assembling the output.

Loop order is ko-outer / gate-inner: per 128-row k-chunk, all 4 gates x 2
batch-tiles accumulate into all 8 PSUM banks.  This lets the PE start as
soon as the first z chunk lands and pipeline directly behind the z stream
(per 2-ko group: ~3.4 us of matmul vs ~2.2 us of DMA), instead of waiting
for the full z load as a gate-outer order would.  z+w_oc0 stream on one
HWDGE ring in 2-ko fused chunks (the ~0.6 us/dma_start issue rate makes
per-ko 3-dma interleaves issue-bound); later w tiles follow whole on the
same ring; bias/c ride a second ring.
"""

import numpy as np
import ml_dtypes

import concourse.bass as bass
import concourse.tile as tile
from concourse import bacc
from concourse import mybir
from concourse.bass_utils import run_bass_kernel_spmd

P = 128
B_FULL, IN, OUT = 4096, 1024, 1024
K = IN + OUT                 # 2048 contraction
RB, RO = 4, 2                # batch-shards x out-col-shards = 8 cores
B_L = B_FULL // RB           # 1024 batch rows per core
O_L = OUT // RO              # 512 out cols per core
KO = K // P                  # 16 k-chunks
OC = O_L // P                # 4 out chunks per core
NG = 4                       # gates
NT = 512                     # moving free dim per matmul (one PSUM bank)
NB = B_L // NT               # 2 batch tiles

F32 = mybir.dt.float32
BF16 = mybir.dt.bfloat16
GATES = ("f", "i", "c", "o")

# exec time of the most recent traced run (ns); set by _run when trace=True
last_exec_time_ns = None

_NC_CACHE = {}


def _build_nc(loop_r=None):
    # loop_r: timing-only variant that repeats the whole body in a hardware
    # For_i loop, so per-iteration device time can be measured as a wall-clock
    # slope between two loop_r values (the per-exec RPC overhead is ~90 ms,
    # far above the kernel time).
    # Bacc (not plain Bass): its compile() pipeline runs
    # move_matmul_waits_to_ldweights + generate_event_semaphores, which split
    # multi-wait instructions to satisfy the TRN2 1-wait-per-inst constraint.
    nc = bacc.Bacc()

    zT = nc.dram_tensor("zT", [K, B_L], BF16, kind="ExternalInput")
    cT = nc.dram_tensor("cT", [O_L, B_L], F32, kind="ExternalInput")
    # gate-fused weights: [k, oc, gate, p] with o_local = oc*128 + p
    wA = nc.dram_tensor("wA", [K, OC, NG, P], BF16, kind="ExternalInput")
    # gate-fused biases: [p, oc, gate]
    bA = nc.dram_tensor("bA", [P, OC, NG], F32, kind="ExternalInput")
    hT = nc.dram_tensor("hT", [O_L, B_L], F32, kind="ExternalOutput")

    zT_t = zT[:, :].rearrange("(ko kp) b -> kp ko b", kp=P)    # [128,16,1024]
    cT_t = cT[:, :].rearrange("(oc p) b -> p oc b", p=P)       # [128,4,1024]
    hT_t = hT[:, :].rearrange("(oc p) b -> p oc b", p=P)
    wA_t = wA[:, :, :, :].rearrange(
        "(ko kp) oc g p -> kp ko oc (g p)", kp=P
    )                                                          # [128,16,4,512]

    sig = mybir.ActivationFunctionType.Sigmoid
    tanh = mybir.ActivationFunctionType.Tanh

    import contextlib

    with tile.TileContext(nc) as tc:
        with (
            tc.For_i(0, loop_r, 1) if loop_r else contextlib.nullcontext(),
            tc.tile_pool(name="zpool", bufs=1) as zpool,
            tc.tile_pool(name="cpool", bufs=2) as cpool,
            tc.tile_pool(name="bpool", bufs=1) as bpool,
            tc.tile_pool(name="wpool", bufs=3) as wpool,
            tc.tile_pool(name="gates", bufs=1) as gpool,
            tc.tile_pool(name="temps", bufs=2) as tpool,
            tc.tile_pool(name="psum", bufs=8, space="PSUM") as psum_pool,
        ):
            z_sb = zpool.tile([P, KO, B_L], BF16)          # 4 MiB resident
            w_tiles = [
                wpool.tile([P, KO, NG * P], BF16, tag="w", name=f"w_oc{oc}")
                for oc in range(OC)
            ]
            c_tiles = [
                cpool.tile([P, B_L], F32, tag="c", name=f"c_oc{oc}")
                for oc in range(OC)
            ]

            # Ring A (sync): the PE-critical stream — fused 2-ko z chunks
            # interleaved with the matching w_oc0 slices, then the whole
            # w_oc1..3 tiles.  Ring B (scalar): bias + c chunks.
            b_sb = bpool.tile([P, OC, NG], F32)
            nc.scalar.dma_start(b_sb[:, :, :], bA[:, :, :])
            # halve the very first chunk so the first matmul starts sooner
            nc.sync.dma_start(z_sb[:, 0:1, :], zT_t[:, 0:1, :])
            nc.sync.dma_start(w_tiles[0][:, 0:1, :], wA_t[:, 0:1, 0, :])
            nc.sync.dma_start(z_sb[:, 1:2, :], zT_t[:, 1:2, :])
            nc.sync.dma_start(w_tiles[0][:, 1:2, :], wA_t[:, 1:2, 0, :])
            for j in range(1, KO // 2):
                ks = slice(2 * j, 2 * j + 2)
                nc.sync.dma_start(z_sb[:, ks, :], zT_t[:, ks, :])
                nc.sync.dma_start(w_tiles[0][:, ks, :], wA_t[:, ks, 0, :])
            for oc in range(1, OC):
                nc.sync.dma_start(w_tiles[oc][:, :, :], wA_t[:, :, oc, :])
            for oc in range(OC):
                nc.scalar.dma_start(c_tiles[oc][:, :], cT_t[:, oc, :])

            for oc in range(OC):
                w_sb = w_tiles[oc]
                c_sb = c_tiles[oc]

                ps = {
                    (gi, nb): psum_pool.tile([P, NT], F32, tag="ps", name="ps")
                    for gi in range(NG)
                    for nb in range(NB)
                }
                for ko in range(KO):
                    for gi in range(NG):
                        for nb in range(NB):
                            nc.tensor.matmul(
                                ps[(gi, nb)][:, :],
                                lhsT=w_sb[:, ko, gi * P:(gi + 1) * P],
                                rhs=z_sb[:, ko, nb * NT:(nb + 1) * NT],
                                start=(ko == 0),
                                stop=(ko == KO - 1),
                            )

                gate_sb = {}
                for gi, g in enumerate(GATES):
                    func = tanh if g == "c" else sig
                    for nb in range(NB):
                        gt = gpool.tile(
                            [P, NT], F32, tag=f"gate_{g}_{nb}",
                            name=f"gate_{g}_{nb}",
                        )
                        nc.scalar.activation(
                            gt[:, :], ps[(gi, nb)][:, :], func,
                            bias=b_sb[:, oc, gi:gi + 1],
                        )
                        gate_sb[(g, nb)] = gt

                for nb in range(NB):
                    bsl = slice(nb * NT, (nb + 1) * NT)
                    cf = tpool.tile([P, NT], F32, tag="cf", name=f"cf_{nb}")
                    nc.vector.tensor_mul(
                        cf[:, :], c_sb[:, bsl], gate_sb[("f", nb)][:, :]
                    )
                    ig = tpool.tile([P, NT], F32, tag="ig", name="ig")
                    nc.vector.tensor_mul(
                        ig[:, :], gate_sb[("i", nb)][:, :],
                        gate_sb[("c", nb)][:, :],
                    )
                    nc.vector.tensor_add(cf[:, :], cf[:, :], ig[:, :])
                    nc.scalar.activation(cf[:, :], cf[:, :], tanh)
                    nc.vector.tensor_mul(
                        cf[:, :], cf[:, :], gate_sb[("o", nb)][:, :]
                    )
                    nc.sync.dma_start(hT_t[:, oc, bsl], cf[:, :])

    # run the Bacc pass pipeline (alloc_regs, wait-splitting, ...);
    # run_bass_via_pjrt does not finalize on our behalf
    nc.finalize()
    return nc


def _get_nc():
    if "nc" not in _NC_CACHE:
        _NC_CACHE["nc"] = _build_nc()
    return _NC_CACHE["nc"]


def _shard_inputs(x, h, c, w_f, b_f, w_i, b_i, w_c, b_c, w_o, b_o):
    ws = {"f": w_f, "i": w_i, "c": w_c, "o": w_o}
    bz = {"f": b_f, "i": b_i, "c": b_c, "o": b_o}
    f32 = np.float32
    bf16 = ml_dtypes.bfloat16

    # per-out-group fused weight/bias shards (shared by the 4 batch groups)
    # wA[k, oc, g, p] = w_g[k, j*O_L + oc*P + p]
    wA_sh = {}
    bA_sh = {}
    for j in range(RO):
        cols = slice(j * O_L, (j + 1) * O_L)
        wA_sh[j] = np.ascontiguousarray(
            np.stack(
                [np.asarray(ws[g][:, cols], dtype=f32).reshape(K, OC, P)
                 for g in GATES],
                axis=2,
            ).astype(bf16)
        )
        bA_sh[j] = np.ascontiguousarray(
            np.stack(
                [np.asarray(bz[g], dtype=f32).reshape(-1)[cols].reshape(OC, P).T
                 for g in GATES],
                axis=2,
            )
        )

    in_maps = []
    for i in range(RB):
        rows = slice(i * B_L, (i + 1) * B_L)
        zT = np.ascontiguousarray(
            np.concatenate([x[rows], h[rows]], axis=1).T.astype(bf16)
        )
        for j in range(RO):
            cT = np.ascontiguousarray(
                c[rows, j * O_L:(j + 1) * O_L].T, dtype=f32
            )
            in_maps.append(
                {"zT": zT, "cT": cT, "wA": wA_sh[j], "bA": bA_sh[j]}
            )
    return in_maps


def _run(in_maps, trace=False, trace_cores=None):
    global last_exec_time_ns
    nc = _get_nc()
    res = run_bass_kernel_spmd(
        nc, in_maps, list(range(RB * RO)),
        trace=trace, trace_cores=trace_cores,
    )
    if trace:
        last_exec_time_ns = res.exec_time_ns
    return res.results


def kernel(x, h, c, w_f, b_f, w_i, b_i, w_c, b_c, w_o, b_o):
    in_maps = _shard_inputs(
        x, h, c, w_f, b_f, w_i, b_i, w_c, b_c, w_o, b_o
    )
    results = _run(in_maps)
    out = np.empty((B_FULL, OUT), np.float32)
    for i in range(RB):
        for j in range(RO):
            shard = results[i * RO + j]["hT"]  # [O_L, B_L]
            out[i * B_L:(i + 1) * B_L, j * O_L:(j + 1) * O_L] = shard.T
    return out
